# revision 6
# baseline (speedup 1.0000x reference)
"""Multi-head scaled-dot-product attention (ABSA-style, per-head projections)
on 8 Trainium2 NeuronCores.

Reference computation (per head h, batch b):
    kx = k @ w_kx[h]                    # (512, 96)
    qx = q @ w_qx[h]                    # (512, 96)
    s  = qx @ kx.T / sqrt(96)           # (512, 512)
    a  = softmax(s, axis=-1)
    o  = a @ kx                         # (512, 96)
    out[b, :, h*96:(h+1)*96] = o

Distribution: data-parallel over batch. 32 batches are split 4-per-core over
8 cores; every core holds the full (tiny) weights and computes all 8 heads
for its 4 batches. No collectives needed — the host concatenates the
per-core outputs.

Differences vs the v1 kernel (204-235 us):
  - q-projection stationaries are zero-padded to 128 columns host-side, so
    FWL (fast weight load) engages (NumWeights==128) and the per-matmul
    LDWEIGHTS tax drops.
  - The k-projection is flipped: stationary = kT seq-chunk [128,128] (full
    PE width), moving = all-heads weight block [128, 4*96].  This streams
    36864 columns/batch instead of 49152 (the 96-row stationary waste is
    gone) and lands kx in NATURAL layout, which is what the attention
    weighted-sum needs — so the old kx re-transpose chain is replaced by a
    per-head transpose that produces kx^T for the scores.
  - Software pipelining: batch b's attention phase is emitted interleaved
    with batch b+1's q-projection (scores(h) -> qproj(b+1,h) -> attn(h-1)),
    so the PE has work while the scalar engine computes exp() — the exp
    chain (4 x 819 ns per head) is the attention-phase critical path.
  - PSUM budget: shared pool (q-proj + scores, 3 bufs), k-proj 2, attn
    output 1 (per-head [128,4,97] tile), transposes 2 = 8 banks.
  - All PSUM evictions on the vector engine; scalar does exp() only.

Measured on 8 axon-tunneled TRN2 NeuronCores (trace runs): see test log.
"""

import math
from functools import lru_cache

import numpy as np

import concourse.bass as bass
import concourse.tile as tile
from concourse import mybir
from concourse.bass_utils import run_bass_kernel_spmd
from concourse.masks import make_identity

# ---------------------------------------------------------------------------
# Workaround for walrus "Too many sync wait commands": some instruction
# encodings accept only a single sync-wait, but Tile can attach several.
# Hoist every wait beyond the first onto a same-engine no-op inserted right
# before the instruction — program order on the engine makes that equivalent.
# ---------------------------------------------------------------------------

import bass_rust as _bass_rust


def _split_excess_waits(nc, max_waits=1):
    n = 0
    for f in nc.m.functions:
        for bb in f.blocks:
            il = bb.instructions
            i = 0
            while i < len(il):
                ins = il[i]
                si = ins.sync_info
                waits = list(si.on_wait or []) if si is not None else []
                if len(waits) > max_waits:
                    si.on_wait = waits[:max_waits]
                    for w in waits[max_waits:]:
                        nop = mybir.InstNoOp(name=f"waitnop-{n}", ins=[],
                                             outs=[])
                        n += 1
                        nop.engine = ins.engine
                        nop.sync_info = _bass_rust.SyncInfo(
                            on_wait=[w], on_update=[])
                        il.insert(i, nop)
                        i += 1
                i += 1

# ---------------------------------------------------------------------------
# Problem constants (full problem; hardcoded per the harness contract)
# ---------------------------------------------------------------------------
EMBED = 768
HID = 96
N_HEAD = 8
BATCH = 32
SEQ = 512
N_CORES = 8
B = BATCH // N_CORES  # batches per core
EC = EMBED // 128  # embed chunks of 128
KC = SEQ // 128  # key (seq) chunks of 128
QC = SEQ // 128  # query chunks of 128
SCALE = 1.0 / math.sqrt(HID)
HP = HID + 1  # per-head kxo stride: 96 data cols + 1 ones col

F32 = mybir.dt.float32
BF16 = mybir.dt.bfloat16


def build_bass():
    nc = bass.Bass("TRN2", target_bir_lowering=False, debug=False,
                   num_devices=N_CORES)

    k_in = nc.declare_dram_parameter("k", [B, SEQ, EMBED], F32, isOutput=False)
    q_in = nc.declare_dram_parameter("q", [B, SEQ, EMBED], F32, isOutput=False)
    # host-packed weights:
    #   w_kx: [128, EC, N_HEAD*HID]  (p, ec, h*96+d) = w_kx[h, ec*128+p, d]
    #   w_qx: [128, N_HEAD*EC, 128]  (p, h*6+ec, d)  = w_qx[h, ec*128+p, d],
    #         d-padded 96->128 with zeros (FWL wants 128 weight columns)
    wk_in = nc.declare_dram_parameter("w_kx", [128, EC, N_HEAD * HID], F32,
                                      isOutput=False)
    wq_in = nc.declare_dram_parameter("w_qx", [128, N_HEAD * EC, 128], F32,
                                      isOutput=False)
    out_d = nc.declare_dram_parameter("out", [B, SEQ, EMBED], F32,
                                      isOutput=True)

    with nc.allow_low_precision("bf16 compute, f32 accumulate"), \
            tile.TileContext(nc) as tc:
        with tc.tile_pool(name="singles", bufs=1) as singles, \
                tc.tile_pool(name="nat", bufs=4) as nat_pool, \
                tc.tile_pool(name="kqt", bufs=1) as kqt_pool, \
                tc.tile_pool(name="wsb", bufs=1) as w_pool, \
                tc.tile_pool(name="stage", bufs=1) as stage_pool, \
                tc.tile_pool(name="exp", bufs=8) as exp_pool, \
                tc.tile_pool(name="recip", bufs=8) as recip_pool, \
                tc.tile_pool(name="ps_qs", bufs=3, space="PSUM") as ps_qs, \
                tc.tile_pool(name="ps_k", bufs=2, space="PSUM") as ps_k, \
                tc.tile_pool(name="ps_att", bufs=1, space="PSUM") as ps_att, \
                tc.tile_pool(name="ps_tr", bufs=2, space="PSUM") as ps_tr:

            # --- one-time setup -------------------------------------------
            identity = singles.tile([128, 128], BF16, tag="identity")
            make_identity(nc, identity[:])

            # --- input pipeline -------------------------------------------
            # SWDGE cast-DMAs (f32 -> bf16, contiguous descriptors).
            # Emission order puts q0 + w_qx first so the PE can start early.
            wq_sb = w_pool.tile([128, N_HEAD * EC, 128], BF16, tag="wq",
                                name="wq_sb")
            wk_sb = w_pool.tile([128, EC, N_HEAD * HID], BF16, tag="wk",
                                name="wk_sb")

            def load_wq(half):
                hb = N_HEAD * EC // 2
                sl = slice(half * hb, (half + 1) * hb)
                nc.gpsimd.dma_start(out=wq_sb[:, sl, :], in_=wq_in[:, sl, :])

            def load_wk():
                nc.gpsimd.dma_start(out=wk_sb[:], in_=wk_in[:])

            def cast_batch_tensor(b, t):
                src_d = (k_in, q_in)[t]
                nat = nat_pool.tile([128, KC, EMBED], BF16,
                                    tag=f"nat{t}", name=f"nat{t}_{b}")
                nc.gpsimd.dma_start(
                    out=nat[:],
                    in_=src_d[b].rearrange("(kc p) e -> p kc e", p=128))
                return nat

            # PE warm-up transposes: fill the startup window (input casts in
            # flight) so the HAM clock gate flips to 2.4 GHz before the real
            # matmuls, and never re-throttles.
            def warmup(n):
                warm_ps = ps_qs.tile([128, 256], BF16, tag="qs",
                                     name="warm_ps")
                for _ in range(n):
                    nc.tensor.transpose(warm_ps[:, 0:128], identity[:],
                                        identity[:])

            # kT/qT (embed on partitions) built with PE transposes.
            kT = {}
            qT = {}

            def input_transpose_one(b, t, nat, ec):
                dst = qT if t else kT
                tp = ps_tr.tile([128, KC, 128], BF16, tag="tr",
                                name="in_tr")
                for kc in range(KC):
                    nc.tensor.transpose(
                        tp[:, kc, :],
                        nat[:, kc, ec * 128:(ec + 1) * 128],
                        identity[:])
                tt = kqt_pool.tile([128, SEQ], BF16,
                                   tag=f"T{t}_{b}_{ec}",
                                   name=f"T{t}_{b}_{ec}")
                nc.vector.tensor_copy(tt[:], tp[:])
                dst[b, ec] = tt

            def input_transposes(b, t, nat):
                for ec in range(EC):
                    input_transpose_one(b, t, nat, ec)

            # --- persistent SBUF staging ----------------------------------
            # qxT / kxT: per (parity, head) [96, 512] bf16 (hid on parts).
            qxT = [[singles.tile([HID, SEQ], BF16, tag=f"qxT_{i}_{h}",
                                 name=f"qxT_{i}_{h}")
                    for h in range(N_HEAD)] for i in range(2)]
            kxT = [[singles.tile([HID, SEQ], BF16, tag=f"kxT_{i}_{h}",
                                 name=f"kxT_{i}_{h}")
                    for h in range(N_HEAD)] for i in range(2)]
            # kxo: per (parity, seq-chunk) [128, N_HEAD, 97] bf16 — kx in
            # natural layout, heads on a 97-stride with a ones column at 96
            # (softmax denominator folded into the attention matmul).
            kxo = [[singles.tile([128, N_HEAD, HP], BF16,
                                 tag=f"kxo_{i}_{sc}", name=f"kxo_{i}_{sc}")
                    for sc in range(KC)] for i in range(2)]
            # output staging: per (parity, q chunk) [128, EMBED] f32.
            stage = [[stage_pool.tile([128, EMBED], F32, tag=f"st{p}_{qc}",
                                      name=f"st{p}_{qc}")
                      for qc in range(QC)] for p in range(2)]

            # --- phase building blocks ------------------------------------
            def qproj_head(b, h):
                # qx^T[h] via padded stationary wq chunk [128,128]:
                # out rows 0:96 = qx^T, rows 96:128 = zeros (pad).
                par = b % 2
                qp = ps_qs.tile([128, SEQ], F32, tag="qs", name="qproj_ps")
                for ec in range(EC):
                    nc.tensor.matmul(qp[:], wq_sb[:, h * EC + ec, :],
                                     qT[b, ec][:],
                                     start=(ec == 0), stop=(ec == EC - 1))
                nc.vector.tensor_copy(qxT[par][h][:], qp[0:HID, :])

            def kproj_chain(b, sc, half):
                # flipped projection: stationary kT seq-chunk [128,128],
                # moving = 4-head weight block [128, 384]; lands kx natural.
                par = b % 2
                dst = kxo[par][sc]
                hs = slice(half * 4 * HID, (half + 1) * 4 * HID)
                kp = ps_k.tile([128, 4, HID], F32, tag="kp",
                               name="kproj_ps")
                for ec in range(EC):
                    nc.tensor.matmul(
                        kp[:],
                        kT[b, ec][:, sc * 128:(sc + 1) * 128],
                        wk_sb[:, ec, hs],
                        start=(ec == 0), stop=(ec == EC - 1))
                nc.vector.tensor_copy(
                    dst[:, half * 4:(half + 1) * 4, 0:HID], kp[:])
                if half == 1:
                    nc.gpsimd.memset(dst[:, :, HID:HP], 1.0)

            def kxT_transpose_head(b, h):
                par = b % 2
                tp = ps_tr.tile([HID, SEQ], BF16, tag="tr", name="kxT_tr")
                for sc in range(KC):
                    nc.tensor.transpose(
                        tp[:, sc * 128:(sc + 1) * 128],
                        kxo[par][sc][:, h, 0:HID],
                        identity[:])
                nc.vector.tensor_copy(kxT[par][h][:], tp[:])

            def scores_exp(b, h):
                # s^T (k on psum partitions), exp folded on eviction.
                par = b % 2
                exp_sb = []
                for sc in range(KC):
                    s_ps = ps_qs.tile([128, SEQ], F32, tag="qs",
                                      name="score_ps")
                    nc.tensor.matmul(
                        s_ps[:], kxT[par][h][:, sc * 128:(sc + 1) * 128],
                        qxT[par][h][:], start=True, stop=True)
                    e_sb = exp_pool.tile([128, SEQ], BF16, tag="exp",
                                         name="e_sb")
                    nc.scalar.activation(
                        e_sb[:], s_ps[:],
                        mybir.ActivationFunctionType.Exp, scale=SCALE)
                    exp_sb.append(e_sb)
                return exp_sb

            def attn_head(b, h, exp_sb):
                # attention-weighted values + softmax denominator (col 96),
                # one [128, 4, 97] psum tile per head (all 4 q chunks).
                par = b % 2
                st = stage[par]
                o_ps = ps_att.tile([128, QC, HP], F32, tag="att",
                                   name="o_ps")
                for qc in range(QC):
                    for kc in range(KC):
                        nc.tensor.matmul(
                            o_ps[:, qc, :],
                            exp_sb[kc][:, qc * 128:(qc + 1) * 128],
                            kxo[par][kc][:, h, :],
                            start=(kc == 0), stop=(kc == KC - 1))
                for qc in range(QC):
                    rc = recip_pool.tile([128, 1], F32, tag="recip",
                                         name="recip")
                    nc.vector.reciprocal(rc[:], o_ps[:, qc, HID:HP])
                    nc.vector.tensor_scalar_mul(
                        st[qc][:, h * HID:(h + 1) * HID],
                        o_ps[:, qc, 0:HID], rc[:])

            def store_half(b, half):
                par = b % 2
                sl = slice(half * (EMBED // 2), (half + 1) * (EMBED // 2))
                for qc in range(QC):
                    nc.sync.dma_start(
                        out=out_d[b, qc * 128:(qc + 1) * 128, sl],
                        in_=stage[par][qc][:, sl])

            # --- emission ----------------------------------------------
            # Startup: casts ordered q0, wq, k0, wk, then batches 1..3;
            # warm-ups bridge the PE until q0 lands.
            nat_q0 = cast_batch_tensor(0, 1)
            load_wq(0)
            nat_k0 = cast_batch_tensor(0, 0)
            load_wq(1)
            load_wk()
            nats = {(0, 1): nat_q0, (0, 0): nat_k0}
            for b in range(1, B):
                for t in (1, 0):
                    nats[(b, t)] = cast_batch_tensor(b, t)

            warmup(56)

            # batch 0 prologue (no previous batch to hide behind)
            input_transposes(0, 1, nats[(0, 1)])
            for h in range(N_HEAD):
                qproj_head(0, h)
            input_transposes(0, 0, nats[(0, 0)])

            for b in range(B):
                # PHASE K(b): k-side projection interleaved with batch
                # b+1's input transposes (PE work hides the DVE eviction
                # latency; the scalar engine drains exp(b-1) meanwhile),
                # then per-head kx^T transposes, h0 first.
                it_chains = []
                if b + 1 < B:
                    it_chains = [(b + 1, t, nats[(b + 1, t)], ec)
                                 for t in (1, 0) for ec in range(EC)]
                ic = 0
                for sc in range(KC):
                    for half in range(2):
                        kproj_chain(b, sc, half)
                        # ~1.5 transpose chains between kproj chains
                        take = 2 if (sc * 2 + half) % 2 else 1
                        for _ in range(take):
                            if ic < len(it_chains):
                                input_transpose_one(*it_chains[ic])
                                ic += 1
                while ic < len(it_chains):
                    input_transpose_one(*it_chains[ic])
                    ic += 1
                for h in range(N_HEAD):
                    kxT_transpose_head(b, h)

                # PHASE A(b): scores -> exp -> attention, with batch b+1's
                # q-projection interleaved so the PE stays busy during exp.
                # attn lags scores by one head (exp latency cover).
                prev = None
                for h in range(N_HEAD):
                    if b + 1 < B:
                        qproj_head(b + 1, h)
                    exp_sb = scores_exp(b, h)
                    if prev is not None:
                        attn_head(b, prev[0], prev[1])
                        if prev[0] == N_HEAD // 2 - 1:
                            store_half(b, 0)
                    prev = (h, exp_sb)
                attn_head(b, prev[0], prev[1])
                store_half(b, 1)

    _split_excess_waits(nc)
    return nc


@lru_cache(maxsize=1)
def _get_nc():
    return build_bass()


def _pack_weights(w_kx, w_qx):
    # w_kx [8, 768, 96] -> [128, 6, 768]: (p, ec, h*96+d)
    wk = np.ascontiguousarray(
        w_kx.reshape(N_HEAD, EC, 128, HID).transpose(2, 1, 0, 3)
        .reshape(128, EC, N_HEAD * HID), dtype=np.float32)
    # w_qx [8, 768, 96] -> [128, 48, 128] zero-padded: (p, h*6+ec, d)
    wq = np.zeros((128, N_HEAD, EC, 128), dtype=np.float32)
    wq[:, :, :, 0:HID] = w_qx.reshape(N_HEAD, EC, 128, HID).transpose(
        2, 0, 1, 3)
    wq = np.ascontiguousarray(wq.reshape(128, N_HEAD * EC, 128))
    return wk, wq


def kernel(k, q, w_kx, w_qx):
    k = np.ascontiguousarray(k, dtype=np.float32)
    q = np.ascontiguousarray(q, dtype=np.float32)
    w_kx = np.ascontiguousarray(w_kx, dtype=np.float32)
    w_qx = np.ascontiguousarray(w_qx, dtype=np.float32)
    wk_packed, wq_packed = _pack_weights(w_kx, w_qx)

    nc = _get_nc()
    in_maps = []
    for c in range(N_CORES):
        sl = slice(c * B, (c + 1) * B)
        in_maps.append({
            "k": np.ascontiguousarray(k[sl]),
            "q": np.ascontiguousarray(q[sl]),
            "w_kx": wk_packed,
            "w_qx": wq_packed,
        })
    res = run_bass_kernel_spmd(nc, in_maps, core_ids=list(range(N_CORES)))
    return np.concatenate([res.results[c]["out"] for c in range(N_CORES)],
                          axis=0)
